# revision 36
# baseline (speedup 1.0000x reference)
"""RBF-kernel causal attention on 8 Trainium2 NeuronCores.

B=2, H=16, N=2048, D=64. Shards the 32 (b,h) attention instances across 8
cores (4 heads per core). Math notes:

  logits = -relu(||q-k||^2)/sqrt(D); relu is a no-op (||q-k||^2 >= 0 up to
  rounding), and softmax is invariant to per-query offsets, so
      softmax_n(-(qsq_m + ksq_n - 2 qk)/8) == softmax_n(qk/4 - ksq_n/8)
  We compute pg = exp(0.25 * K Q^T) in a [key, query] layout (bf16) and fold
  the exp(-0.125 ksq_n) per-key factor into V (and into the appended
  ones-column that produces the softmax denominator):
      O_aug[q, 0:65] accumulates via matmul(lhsT=pg_slice, rhs=V_aug_scaled)
  directly in the natural [query, feature] layout, so no output transpose is
  needed. Final O[q, d] = O_aug[q, d] / O_aug[q, 64].

Engine plan (per core): ACT does all the exps (the ~80us bottleneck; exp has
no second engine on TRN2); PE does QK (row-packed bf16 halves, 1 cyc/row)
+ PV; the DMA engines do the K^T/Q^T transposes (batched dma_start_transpose,
14ns per 16x128 xbar tile) instead of PE transposes + DVE staging copies;
Pool does the f32->bf16 converts; DVE does ksq, causal-mask muls (bf16 2x
mode), and the epilogue divide.

Keys and queries are pair-interleaved in SBUF (partition p of a 256-row
chunk holds rows {2p, 2p+1}) so every DMA descriptor moves 512B contiguous
(2x fewer descriptors); softmax is permutation-invariant over keys, and the
causal masks / store patterns account for the query permutation.

Scheduling notes (hard-won, from the timeline-sim cost model):
- PSUM accumulation chains that time-interleave within one bank corrupt
  each other; chains in different banks, or strictly sequential chains in
  one bank, are exact. Hence PV runs chain-major (one query tile at a
  time) per job, deferred by one job so the matmuls never wait at the head
  of PE's in-order queue.
- A reader of a multi-writer tile can get a conservative wait on ALL its
  writers; everything in the setup path is therefore a per-piece
  (2-key-chunk) single-writer tile.
- Each engine's sequencer is in-order: the w-exps are emitted after the
  transfer pieces so they don't gate the main exps on ACT, and head 0
  loads with one DMA per tensor so the transposes aren't stuck behind a
  dozen 650ns HWDGE slots on SP.
"""

import sys

if "/opt/trn_rl_repo" not in sys.path:
    sys.path.insert(0, "/opt/trn_rl_repo")

import numpy as np

import concourse.bacc as bacc
import concourse.mybir as mybir
import concourse.tile as tile

B, H, N, D = 2, 16, 2048, 64
NCORES = 8
HPC = (B * H) // NCORES  # heads per core = 4
P = 128                  # partitions
CH = N // 256            # 256-key chunks per head = 8
QB = 512                 # query block
MBS = N // QB            # query blocks per head = 4
NT = 2 * CH              # 128-key tiles per head = 16 (chunk t, parity r)

F32 = mybir.dt.float32
BF16 = mybir.dt.bfloat16
EXP = mybir.ActivationFunctionType.Exp


def build_nc():
    nc = bacc.Bacc("TRN2", target_bir_lowering=False, debug=False)
    q = nc.dram_tensor("q", [HPC, N, D], F32, kind="ExternalInput")
    k = nc.dram_tensor("k", [HPC, N, D], F32, kind="ExternalInput")
    v = nc.dram_tensor("v", [HPC, N, D], F32, kind="ExternalInput")
    out = nc.dram_tensor("out", [HPC, N, D], F32, kind="ExternalOutput")

    with tile.TileContext(nc) as tc:
        with (
            tc.tile_pool(name="const", bufs=1) as const_pool,
            tc.tile_pool(name="loads", bufs=1) as load_pool,
            tc.tile_pool(name="head", bufs=2) as head_pool,
            tc.tile_pool(name="pg", bufs=20) as pg_pool,
            tc.tile_pool(name="epi", bufs=3) as epi_pool,
            tc.tile_pool(name="stg", bufs=3, space="PSUM") as stg_pool,
            tc.tile_pool(name="ob", bufs=2, space="PSUM") as ob_pool,
        ):
            # causal mask for the diagonal 256-key chunk vs its own 256
            # queries, in the pair-interleaved (key = 2p + r, query =
            # 2p' + r') coordinates: M[p, r, r'*128 + p'] = 1 iff
            # 2p' + r' >= 2p + r.
            M = const_pool.tile([P, 2, 256], BF16, name="mask")
            nc.gpsimd.memset(M[:], 1.0)
            for r in range(2):
                for rp in range(2):
                    nc.gpsimd.affine_select(
                        out=M[:, r, 128 * rp : 128 * rp + 128],
                        in_=M[:, r, 128 * rp : 128 * rp + 128],
                        compare_op=mybir.AluOpType.is_ge, fill=0.0,
                        base=rp - r, pattern=[[2, P]], channel_multiplier=-2,
                    )


            # per-head input tiles; the DMA loads are emitted inside
            # setup_chunks so each head's transposes don't queue behind
            # later heads' loads on SP's in-order sequencer.
            # Pair-interleaved: nat[p, t, r, d] = x[256t + 2p + r, d] so each
            # descriptor is 512B ((r, d) contiguous in DRAM).
            knats, qnats, vnats = [], [], []
            for h in range(HPC):
                knats.append(load_pool.tile([P, CH, 2, D], F32, tag=f"knat{h}", name="kn"))
                qnats.append(load_pool.tile([P, CH, 2, D], F32, tag=f"qnat{h}", name="qn"))
                vnats.append(load_pool.tile([P, CH, 2, D], F32, tag=f"vnat{h}", name="vn"))

            heads = [{} for _ in range(HPC)]

            def setup_chunks(h):
                """Emission chunks for head h's setup, in dependency order."""
                st = heads[h]

                def allocs():
                    # everything is allocated per 2-chunk piece: single-writer
                    # tiles keep the scheduler's RAW waits precise (a reader
                    # of a multi-writer tile waits for ALL its writers)
                    npc = CH // 2
                    st["kbf"] = [
                        head_pool.tile([P, 2, 2, D], BF16, tag=f"kbf{pi}", name="kbf")
                        for pi in range(npc)
                    ]
                    st["qbfd"] = [
                        head_pool.tile([P, 2, 2, 2, D], BF16, tag=f"qbfd{pi}", name="qbfd")
                        for pi in range(npc)
                    ]
                    # kt[:, t, :]: partitions r*64+d hold K^T of parity-r keys
                    # of chunk t; free j = key (256t + 2j + r)
                    st["kt"] = [
                        head_pool.tile([P, 2, P], BF16, tag=f"kt{pi}", name="kt")
                        for pi in range(npc)
                    ]
                    # qt[:, i, :]: Q^T of query tile i=(t', r'), duplicated on
                    # both partition halves (for the row-packed QK matmuls)
                    st["qt"] = [
                        head_pool.tile([P, 4, P], BF16, tag=f"qt{pi}", name="qt")
                        for pi in range(npc)
                    ]
                    st["ksq"] = [
                        head_pool.tile([P, 4], F32, tag=f"ksq{pi}", name="ksq")
                        for pi in range(npc)
                    ]
                    st["w"] = [
                        head_pool.tile([P, 4], F32, tag=f"w{pi}", name="w")
                        for pi in range(npc)
                    ]
                    st["vaug"] = [
                        head_pool.tile([P, 4, D + 1], BF16, tag=f"vaug{pi}", name="vaug")
                        for pi in range(npc)
                    ]

                def loads(c0, cw):
                    eng = nc.sync
                    def run():
                        cs = slice(c0, c0 + cw)
                        eng.dma_start(
                            knats[h][:, cs],
                            k[h].rearrange("(t p r) d -> p t r d", p=P, r=2)[:, cs],
                        )
                        eng.dma_start(
                            qnats[h][:, cs],
                            q[h].rearrange("(t p r) d -> p t r d", p=P, r=2)[:, cs],
                        )
                        eng.dma_start(
                            vnats[h][:, cs],
                            v[h].rearrange("(t p r) d -> p t r d", p=P, r=2)[:, cs],
                        )

                    return run

                def piece(pi):
                    # one 2-chunk piece of the convert + transpose pipeline
                    def run():
                        cs = slice(2 * pi, 2 * pi + 2)
                        kbf, qbfd = st["kbf"][pi], st["qbfd"][pi]
                        knat, qnat = knats[h], qnats[h]
                        nc.gpsimd.tensor_copy(out=kbf[:], in_=knat[:, cs])
                        nc.gpsimd.tensor_copy(
                            out=qbfd[:],
                            in_=qnat[:, cs, :, None, :].to_broadcast(
                                (P, 2, 2, 2, D)
                            ),
                        )
                        nc.sync.dma_start_transpose(
                            st["kt"][pi][:],
                            kbf[:].rearrange("p a b c -> p (a b c)"),
                        )
                        nc.sync.dma_start_transpose(
                            st["qt"][pi][:],
                            qbfd[:].rearrange("p a b c d -> p (a b c d)"),
                        )

                    return run

                def waug(pi):
                    # ksq -> w -> vaug for the piece's 4 key tiles. Emitted
                    # AFTER the head's transfer pieces (dripped into the job
                    # stream): the w-exp would otherwise sit at the head of
                    # ACT's in-order queue waiting on the DVE ksq chain and
                    # gate all the main exps behind it.
                    def run():
                        cs = slice(2 * pi, 2 * pi + 2)
                        knat, vnat = knats[h], vnats[h]
                        ksq, w, vaug_ = st["ksq"][pi], st["w"][pi], st["vaug"][pi]
                        ktmp = epi_pool.tile([P, 2, 2, D], F32, tag="ktmp")
                        nc.vector.tensor_mul(
                            out=ktmp[:], in0=knat[:, cs], in1=knat[:, cs]
                        )
                        nc.vector.tensor_reduce(
                            ksq[:],
                            ktmp[:].rearrange("p a b c -> p (a b) c"),
                            axis=mybir.AxisListType.X, op=mybir.AluOpType.add,
                        )
                        nc.scalar.activation(w[:], ksq[:], EXP, scale=-0.125)
                        nc.vector.tensor_mul(
                            out=vaug_[:, :, :D],
                            in0=vnat[:, cs].rearrange("p a b c -> p (a b) c"),
                            in1=w[:, :, None].to_broadcast((P, 4, D)),
                        )
                        nc.vector.tensor_copy(
                            out=vaug_[:, :, D : D + 1], in_=w[:, :, None]
                        )

                    return run

                yield allocs
                if h == 0:
                    # whole-head loads (one DMA per tensor): fewer 650ns
                    # HWDGE slots on SP's in-order queue before the first
                    # transpose can issue
                    yield loads(0, CH)
                    for pi in range(CH // 2):
                        yield piece(pi)
                    for pi in range(CH // 2):
                        yield waug(pi)
                else:
                    for pi in range(CH // 2):
                        yield piece(pi)
                    for pi in range(CH // 2):
                        yield waug(pi)

            def job_chunks(h, mb):
                """Chunks of one (head, query-block) job.

                QK + exp stream per 256-key chunk; the PV accumulations run
                chain-major at the end of the job (one query tile's full
                accumulation at a time) because interleaving accumulation
                chains within one PSUM bank corrupts them, and this also keeps
                PE's in-order SEQ from stalling on exp-dependent PV matmuls
                between QK chunks.
                """
                kts, qts, vaugs = heads[h]["kt"], heads[h]["qt"], heads[h]["vaug"]
                qt = qts[mb]  # piece mb holds exactly this block's 4 Q^T tiles
                jst = {"pgs": []}

                def sub_chunk(c):
                    def run():
                        stg = stg_pool.tile([P, 2, QB], F32, tag="stg", name="stg")
                        for r in range(2):
                            nc.tensor.matmul(
                                stg[:, r, :],
                                kts[c // 2][64 * r : 64 * r + 64, c % 2, :],
                                qt[64 * r : 64 * r + 64, :, :],
                                start=True, stop=True, skip_group_check=True,
                            )
                        pg = pg_pool.tile([P, 2, QB], BF16, tag="pg")
                        nc.scalar.activation(pg[:], stg[:], EXP, scale=0.25)
                        jst["pgs"].append((pg, c, False))

                    return run

                def diag0():
                    # chunk 2mb: keys [512mb, 512mb+256) vs all 512 queries;
                    # mask applies on query cols 0:256
                    def run():
                        c = 2 * mb
                        stg = stg_pool.tile([P, 2, QB], F32, tag="stg", name="stg")
                        for r in range(2):
                            nc.tensor.matmul(
                                stg[:, r, :],
                                kts[c // 2][64 * r : 64 * r + 64, c % 2, :],
                                qt[64 * r : 64 * r + 64, :, :],
                                start=True, stop=True, skip_group_check=True,
                            )
                        pg = pg_pool.tile([P, 2, QB], BF16, tag="pg")
                        nc.scalar.activation(pg[:], stg[:], EXP, scale=0.25)
                        nc.vector.tensor_mul(
                            out=pg[:, :, 0:256], in0=pg[:, :, 0:256], in1=M[:]
                        )
                        jst["pgs"].append((pg, c, False))

                    return run

                def diag1():
                    # chunk 2mb+1: keys [512mb+256, 512mb+512) vs query cols
                    # 256:512 only (cols 0:256 fully masked, skipped)
                    def run():
                        c = 2 * mb + 1
                        stg = stg_pool.tile([P, 2, QB], F32, tag="stg", name="stg")
                        for r in range(2):
                            nc.tensor.matmul(
                                stg[:, r, 256:],
                                kts[c // 2][64 * r : 64 * r + 64, c % 2, :],
                                qt[64 * r : 64 * r + 64, 2:4, :],
                                start=True, stop=True, skip_group_check=True,
                            )
                        pg = pg_pool.tile([P, 2, QB], BF16, tag="pg")
                        nc.scalar.activation(
                            pg[:, :, 256:], stg[:, :, 256:], EXP, scale=0.25
                        )
                        nc.vector.tensor_mul(
                            out=pg[:, :, 256:], in0=pg[:, :, 256:], in1=M[:]
                        )
                        jst["pgs"].append((pg, c, True))

                    return run

                def pv_epilogue():
                    ob = ob_pool.tile([P, 4, D + 1], F32, tag="ob", name="ob")
                    for i in range(4):
                        rel = [e for e in jst["pgs"] if not (e[2] and i < 2)]
                        for gi, (pg, c, _) in enumerate(rel):
                            for r in range(2):
                                ti = 2 * c + r
                                nc.tensor.matmul(
                                    ob[:, i, :],
                                    pg[:, r, P * i : P * i + P],
                                    vaugs[ti // 4][:, ti % 4, :],
                                    start=(gi == 0 and r == 0),
                                    stop=(gi == len(rel) - 1 and r == 1),
                                    skip_group_check=True,
                                )
                    linv = epi_pool.tile([P, 4], F32, tag="linv")
                    nc.vector.reciprocal(linv[:], ob[:, :, D])
                    o_sb = epi_pool.tile([P, 2, 2, D], F32, tag="o_sb")
                    nc.vector.tensor_mul(
                        out=o_sb[:].rearrange("p a b c -> p (a b) c"),
                        in0=ob[:, :, :D],
                        in1=linv[:, :, None].to_broadcast((P, 4, D)),
                    )
                    nc.sync.dma_start(
                        out[h, mb * QB : (mb + 1) * QB, :].rearrange(
                            "(t p r) d -> p t r d", p=P, r=2
                        ),
                        o_sb[:],
                    )

                chunks = [sub_chunk(c) for c in range(2 * mb)]
                chunks += [diag0(), diag1()]
                return chunks, pv_epilogue

            # ---- software-pipelined emission ----
            # Jobs interleave 2-deep; each job's PV+epilogue is deferred by
            # one job so it never waits at the head of PE's in-order queue
            # (its pg tiles are ready well before it's emitted).
            setup0 = list(setup_chunks(0))
            for c in setup0[: -CH // 2]:
                c()
            pending0 = setup0[-CH // 2 :]
            # prefetch the remaining heads' inputs now: SP is otherwise idle
            # until the first stores, and the transposes for these heads are
            # dripped much later
            for h_ in range(1, HPC):
                for c0 in range(0, CH, 4):
                    cs = slice(c0, c0 + 4)
                    nc.sync.dma_start(
                        knats[h_][:, cs],
                        k[h_].rearrange("(t p r) d -> p t r d", p=P, r=2)[:, cs],
                    )
                    nc.sync.dma_start(
                        qnats[h_][:, cs],
                        q[h_].rearrange("(t p r) d -> p t r d", p=P, r=2)[:, cs],
                    )
                    nc.sync.dma_start(
                        vnats[h_][:, cs],
                        v[h_].rearrange("(t p r) d -> p t r d", p=P, r=2)[:, cs],
                    )
            pending = pending0
            jobs = [(h, mb) for h in range(HPC) for mb in range(MBS)]
            active = []
            deferred = []
            ji = 0
            drip = 0
            while active or ji < len(jobs):
                while len(active) < 2 and ji < len(jobs):
                    h, mb = jobs[ji]
                    if mb == 0 and h > 0 and pending:
                        for c in pending:
                            c()
                        pending = []
                    if mb == 0 and h + 1 < HPC:
                        pending = pending + list(setup_chunks(h + 1))
                    while len(deferred) >= 2:
                        deferred.pop(0)()
                    chunks, pv_fn = job_chunks(h, mb)
                    active.append((chunks, pv_fn))
                    ji += 1
                if ji >= len(jobs) and deferred:
                    deferred.pop(0)()
                for entry in list(active):
                    chunks, pv_fn = entry
                    chunks.pop(0)()
                    drip += 1
                    if drip % 3 == 0 and pending:
                        pending.pop(0)()
                    if not chunks:
                        deferred.append(pv_fn)
                        active.remove(entry)
            for c in pending:
                c()
            for fn in deferred:
                fn()

    nc.compile()
    return nc


_NC = None


def _get_nc():
    global _NC
    if _NC is None:
        _NC = build_nc()
    return _NC


def kernel(q: np.ndarray, k: np.ndarray, v: np.ndarray) -> np.ndarray:
    from concourse.bass_utils import run_bass_kernel_spmd

    nc = _get_nc()
    qf = np.ascontiguousarray(np.asarray(q, dtype=np.float32).reshape(B * H, N, D))
    kf = np.ascontiguousarray(np.asarray(k, dtype=np.float32).reshape(B * H, N, D))
    vf = np.ascontiguousarray(np.asarray(v, dtype=np.float32).reshape(B * H, N, D))
    in_maps = [
        {
            "q": np.ascontiguousarray(qf[c * HPC : (c + 1) * HPC]),
            "k": np.ascontiguousarray(kf[c * HPC : (c + 1) * HPC]),
            "v": np.ascontiguousarray(vf[c * HPC : (c + 1) * HPC]),
        }
        for c in range(NCORES)
    ]
    res = run_bass_kernel_spmd(nc, in_maps, core_ids=list(range(NCORES)))
    outs = [res.results[c]["out"] for c in range(NCORES)]
    return np.concatenate(outs, axis=0).reshape(B, H, N, D)


if __name__ == "__main__":
    rng = np.random.default_rng(0)
    qq = rng.standard_normal((B, H, N, D), dtype=np.float32)
    kk = rng.standard_normal((B, H, N, D), dtype=np.float32)
    vv = rng.standard_normal((B, H, N, D), dtype=np.float32)
    o = kernel(q=qq, k=kk, v=vv)
    print("kernel ran, out shape", o.shape, "finite:", np.isfinite(o).all())
